# revision 22
# baseline (speedup 1.0000x reference)
"""Trainium2 Bass kernel for BbBartAttention (sparse relative-position bias).

Sharding: 8 cores = 4 batches x 2 head-groups (6 heads each).
Each core computes, for its (batch b, head-group g):
  q/k/v projections (transposed layouts), per-head biased attention scores,
  softmax (no max-subtraction; logits are O(1)), PV with a ones-column to
  get softmax denominators, normalization, and a partial output projection
  over its 384 head-dims. Host sums the two group partials per batch and
  adds the (o_b + v_b @ o_w_g.T) row, which is exact because softmax rows
  sum to 1.

Bias structure: with A[i,j] = [pos_row_i==pos_row_j], C[i,j] = [pos_col_i==
pos_col_j], the reference's table index is exactly A + 2*C, so
  bias = t0 + (t1-t0)*A + (t2-t0)*C + (t3-t1-t2+t0)*A*C
A and C fold into the score matmul via one-hot encodings R [64,S], P [32,S]:
one extra matmul with lhsT = [c1*R; c2*P] against rhs = [R; P]. The A*C term
uses D = relu((A+C) - 1) in {0,1}, materialized once per batch from the
[R;P] gram, and added per head via a scaled-identity matmul. t0 rides the
Exp activation's bias operand.

Scores are computed transposed, S^T[j, i] (key index on partitions), so the
softmax denominator falls out of the PV matmul via a ones-column appended to
V, and the attention output lands directly in the [head_dim, i] layout the
output projection needs as lhsT. No on-chip transposes anywhere.
"""

import numpy as np
from contextlib import ExitStack

import ml_dtypes
import concourse.bass as bass
import concourse.tile as tile
from concourse import bacc, mybir
from concourse.bass_utils import run_bass_kernel_spmd

F32 = mybir.dt.float32
F32R = mybir.dt.float32r
BF16 = mybir.dt.bfloat16
AF = mybir.ActivationFunctionType
ALU = mybir.AluOpType

B, S, E, H = 4, 1024, 768, 12
D_HEAD = 64
SCALING = D_HEAD ** -0.5
HG = 2            # head groups (tensor-parallel)
HPG = H // HG     # 6 heads per group
GD = HPG * D_HEAD # 384 head-dims per group
NROW, NCOL = 64, 32
RP = NROW + NCOL  # 96
KT = E // 128     # 6 contraction tiles for projections
MT = GD // 128    # 3 m-tiles for Q^T/K^T
JT = S // 128     # 8 key tiles
IT = S // 512     # 2 query column tiles

_CACHE = {}


def _r(ap):
    return ap.bitcast(F32R)


def build_nc():
    if "nc" in _CACHE:
        return _CACHE["nc"]
    nc = bacc.Bacc("TRN2", target_bir_lowering=False, debug=False, num_devices=8)

    def inp(name, shape, dt=F32):
        return nc.dram_tensor(name, shape, dt, kind="ExternalInput").ap()

    x_hsT = inp("hsT", [E, S], F32R)
    x_wqT = inp("wqT", [E, GD], F32R)
    x_wkT = inp("wkT", [E, GD], F32R)
    x_wvT = inp("wvT", [E, GD], F32R)
    x_woT = inp("woT", [GD, E], F32R)
    x_rpt = inp("rpt", [RP, S], BF16)
    x_eye = inp("eye", [128, 128], BF16)
    x_qb = inp("qb", [128, MT])
    x_kb = inp("kb", [128, MT])
    x_c12 = inp("c12", [RP, HPG])    # rows 0-63 = c1[h], 64-95 = c2[h]
    x_c3 = inp("c3", [128, HPG])
    x_c0 = inp("c0", [128, HPG])
    y_out = nc.dram_tensor("outp", [S, E], F32, kind="ExternalOutput").ap()

    with tile.TileContext(nc) as tc:
        with ExitStack() as ctx:
            cp = ctx.enter_context(tc.tile_pool(name="const", bufs=1))

            # ---- persistent SBUF tensors ----
            rpt = cp.tile([RP, S], BF16, tag="rpt")
            eye = cp.tile([128, 128], BF16, tag="eye")
            qb = cp.tile([128, MT], F32, tag="qb")
            kb = cp.tile([128, MT], F32, tag="kb")
            c12 = cp.tile([RP, HPG], F32, tag="c12")
            c3 = cp.tile([128, HPG], F32, tag="c3")
            c0 = cp.tile([128, HPG], F32, tag="c0")
            wqT = [cp.tile([128, GD], F32R, tag=f"wqT{k}", name=f"wqT{k}") for k in range(KT)]
            wkT = [cp.tile([128, GD], F32R, tag=f"wkT{k}", name=f"wkT{k}") for k in range(KT)]
            wvT = [cp.tile([128, GD], F32R, tag=f"wvT{k}", name=f"wvT{k}") for k in range(KT)]
            woT = [cp.tile([128, E], F32R, tag=f"woT{m}", name=f"woT{m}") for m in range(MT)]
            QT = [cp.tile([128, S], F32R, tag=f"QT{m}", name=f"QT{m}") for m in range(MT)]
            KTt = [cp.tile([128, S], F32R, tag=f"KTt{m}", name=f"KTt{m}") for m in range(MT)]
            # V with interleaved ones column per head: [V_h (64) | 1], 6*65=390
            V = [cp.tile([128, HPG * 65], F32R, tag=f"V{j}", name=f"V{j}") for j in range(JT)]
            Dm = [cp.tile([128, S], BF16, tag=f"D{j}", name=f"Dm{j}") for j in range(JT)]
            rps = [cp.tile([RP, S], BF16, tag=f"rps{h}", name=f"rps{h}") for h in range(HPG)]
            ic3 = [cp.tile([128, 128], BF16, tag=f"ic3{h}", name=f"ic3{h}") for h in range(HPG)]
            xT = [cp.tile([128, S], F32R, tag=f"xT{m}", name=f"xT{m}") for m in range(MT)]

            # ---- phase 1: projections + bias prep ----
            with ExitStack() as p1:
                hp = p1.enter_context(tc.tile_pool(name="hs", bufs=1))
                hsT = [hp.tile([128, S], F32R, tag=f"hsT{k}", name=f"hsT{k}") for k in range(KT)]
                # DMA emission order = priority hint: tiny metadata first
                # (bias prep only needs these), then activations + q/k
                # weights so the first projection matmuls start early
                nc.sync.dma_start(rpt[:], x_rpt)
                nc.sync.dma_start(eye[:], x_eye)
                for t, x in [(qb, x_qb), (kb, x_kb), (c12, x_c12), (c3, x_c3), (c0, x_c0)]:
                    nc.sync.dma_start(t[:], x)
                for k in range(KT):
                    nc.sync.dma_start(hsT[k][:], x_hsT[k * 128:(k + 1) * 128, :])
                    nc.sync.dma_start(wqT[k][:], x_wqT[k * 128:(k + 1) * 128, :])
                    nc.sync.dma_start(wkT[k][:], x_wkT[k * 128:(k + 1) * 128, :])
                for k in range(KT):
                    nc.sync.dma_start(wvT[k][:], x_wvT[k * 128:(k + 1) * 128, :])
                for m in range(MT):
                    nc.sync.dma_start(woT[m][:], x_woT[m * 128:(m + 1) * 128, :])

                ps = p1.enter_context(tc.tile_pool(name="ps1", bufs=2, space="PSUM"))
                # per-head scaled [c1*R; c2*P] and c3*I; D = relu((A+C) - 1)
                for h in range(HPG):
                    nc.vector.tensor_scalar_mul(rps[h][:], rpt[:], c12[:, h:h + 1])
                    nc.vector.tensor_scalar_mul(ic3[h][:], eye[:], c3[:, h:h + 1])
                for j in range(JT):
                    for i2 in range(IT):
                        acc = ps.tile([128, 512], F32, tag="gram")
                        nc.tensor.matmul(
                            acc[:],
                            rpt[:, j * 128:(j + 1) * 128],
                            rpt[:, i2 * 512:(i2 + 1) * 512],
                            start=True, stop=True)
                        nc.vector.tensor_scalar(
                            Dm[j][:, i2 * 512:(i2 + 1) * 512], acc[:],
                            -1.0, 0.0, ALU.add, ALU.max)
                for m in range(MT):
                    for (wt, dst, bias) in [(wqT, QT, qb), (wkT, KTt, kb)]:
                        for i2 in range(IT):
                            acc = ps.tile([128, 512], F32, tag="proj")
                            for k in range(KT):
                                nc.tensor.matmul(
                                    acc[:],
                                    wt[k][:, m * 128:(m + 1) * 128],
                                    hsT[k][:, i2 * 512:(i2 + 1) * 512],
                                    start=(k == 0), stop=(k == KT - 1))
                            nc.scalar.activation(
                                dst[m][:, i2 * 512:(i2 + 1) * 512], acc[:],
                                AF.Identity, bias=bias[:, m:m + 1])
                for j in range(JT):
                    acc = ps.tile([128, GD], F32, tag="projv")
                    for k in range(KT):
                        nc.tensor.matmul(
                            acc[:],
                            hsT[k][:, j * 128:(j + 1) * 128],
                            wvT[k][:],
                            start=(k == 0), stop=(k == KT - 1))
                    # strided copy into [V_h | ones] layout
                    vv = V[j][:].rearrange("p (h c) -> p h c", c=65)
                    av = acc[:].rearrange("p (h c) -> p h c", c=64)
                    nc.vector.tensor_copy(vv[:, :, 0:64], av)
                    # ones column; walrus rejects MEMSET on f32r, so use
                    # (x * 0) + 1 via tensor_scalar instead
                    nc.vector.tensor_scalar(
                        vv[:, :, 64:65], av[:, :, 0:1], 0.0, 1.0,
                        ALU.mult, ALU.add)

            # ---- phase 2: attention per head-pair ----
            with ExitStack() as p2:
                sp = p2.enter_context(tc.tile_pool(name="spsum", bufs=2, space="PSUM"))
                vp = p2.enter_context(tc.tile_pool(name="vpsum", bufs=2, space="PSUM"))
                pp = p2.enter_context(tc.tile_pool(name="probs", bufs=6))
                npl = p2.enter_context(tc.tile_pool(name="norm", bufs=1))
                for p in range(MT):  # head pair p: heads 2p, 2p+1
                    po = [vp.tile([65, S], F32, tag="pv", name="po") for _ in range(2)]
                    for j in range(JT):
                        for hh in range(2):
                            h = 2 * p + hh
                            sm = sp.tile([128, S], F32, tag="s")
                            # same-lhsT matmuls adjacent (QK pair, RP pair,
                            # D pair) to keep the weight path dense
                            for i2 in range(IT):
                                nc.tensor.matmul(
                                    sm[:, i2 * 512:(i2 + 1) * 512],
                                    KTt[p][hh * 64:(hh + 1) * 64, j * 128:(j + 1) * 128],
                                    QT[p][hh * 64:(hh + 1) * 64, i2 * 512:(i2 + 1) * 512],
                                    start=True, stop=False,
                                    tile_position=(hh * 64, 0))
                            for i2 in range(IT):
                                nc.tensor.matmul(
                                    sm[:, i2 * 512:(i2 + 1) * 512],
                                    rps[h][:, j * 128:(j + 1) * 128],
                                    rpt[:, i2 * 512:(i2 + 1) * 512],
                                    start=False, stop=False)
                            for i2 in range(IT):
                                nc.tensor.matmul(
                                    sm[:, i2 * 512:(i2 + 1) * 512],
                                    ic3[h][:],
                                    Dm[j][:, i2 * 512:(i2 + 1) * 512],
                                    start=False, stop=True)
                            pr = pp.tile([128, S], F32R, tag="pr")
                            nc.scalar.activation(
                                pr[:], sm[:], AF.Exp, bias=c0[:, h:h + 1])
                            for i2 in range(IT):
                                nc.tensor.matmul(
                                    po[hh][:, i2 * 512:(i2 + 1) * 512],
                                    V[j][:, h * 65:(h + 1) * 65],
                                    pr[:, i2 * 512:(i2 + 1) * 512],
                                    start=(j == 0), stop=(j == JT - 1))
                    # For pairs 0/1, evict PV psums to SBUF immediately so the
                    # next pair's PV matmuls get the psum slots back, then
                    # normalize off the SBUF copies (off the PE critical
                    # path). The last pair has no successor — normalize
                    # straight from psum to shorten the tail before the
                    # output projection.
                    last = (p == MT - 1)
                    if last:
                        # Tail-latency-optimized: DMA can't read PSUM, so
                        # stage only the den rows in SBUF (one per engine so
                        # they run in parallel); normalize mults read psum
                        # directly, hh=1 via GpSimd which can shift
                        # partitions (software engine), skipping the
                        # round-trip DMA.
                        xo = po
                        dn = [npl.tile([1, S], F32, tag=f"dn{hh}", name="dn",
                                       bufs=1) for hh in range(2)]
                        nc.vector.tensor_copy(dn[0][:], po[0][64:65, :])
                        nc.vector.tensor_copy(dn[1][:], po[1][64:65, :])
                    else:
                        xo = [npl.tile([65, S], F32, tag=f"xo{hh}", name="xo",
                                       bufs=2) for hh in range(2)]
                        for hh in range(2):
                            nc.vector.tensor_copy(xo[hh][:], po[hh][:])
                        dn = [xo[hh][64:65, :] for hh in range(2)]
                    # A [1, S] reciprocal is single-lane-serial on DVE
                    # (~6.5us); bounce both heads' denominator rows through a
                    # [128, 16] layout so all lanes work (~0.2us).
                    rt = npl.tile([128, 16], F32, tag="rt", bufs=2)
                    for hh in range(2):
                        nc.sync.dma_start(rt[:, hh * 8:(hh + 1) * 8],
                                          dn[hh][:])
                    rr = npl.tile([128, 16], F32, tag="rr", bufs=2)
                    nc.vector.reciprocal(rr[:], rt[:])
                    rc = npl.tile([1, 2 * S], F32, tag="rc", bufs=2)
                    for hh in range(2):
                        nc.sync.dma_start(rc[:, hh * S:(hh + 1) * S],
                                          rr[:, hh * 8:(hh + 1) * 8])
                    rb = npl.tile([64, 2 * S], F32, tag="rb", bufs=2)
                    nc.gpsimd.partition_broadcast(rb[:], rc[:])
                    nc.vector.tensor_tensor(
                        xT[p][0:64, :], xo[0][0:64, :], rb[:, 0:S], ALU.mult)
                    # DVE cannot shift partitions (and GpSimd cannot read
                    # PSUM): normalize at base 0 then DMA into 64-127.
                    nm = npl.tile([64, S], F32R, tag="nm", bufs=2)
                    nc.vector.tensor_tensor(
                        nm[:], xo[1][0:64, :], rb[:, S:2 * S], ALU.mult)
                    nc.sync.dma_start(xT[p][64:128, :], nm[:])

            # ---- phase 3: output projection (partial over this group's dims) ----
            with ExitStack() as p3:
                fp = p3.enter_context(tc.tile_pool(name="fpsum", bufs=4, space="PSUM"))
                op = p3.enter_context(tc.tile_pool(name="oev", bufs=4))
                for i8 in range(JT):  # 8 query 128-blocks
                    ev = op.tile([128, E], F32, tag="ev")
                    for n2 in range(2):
                        acc = fp.tile([128, 384], F32, tag=f"f{n2}")
                        for m in range(MT):
                            nc.tensor.matmul(
                                acc[:],
                                xT[m][:, i8 * 128:(i8 + 1) * 128],
                                woT[m][:, n2 * 384:(n2 + 1) * 384],
                                start=(m == 0), stop=(m == MT - 1))
                        nc.scalar.copy(ev[:, n2 * 384:(n2 + 1) * 384], acc[:])
                    nc.sync.dma_start(y_out[i8 * 128:(i8 + 1) * 128, :], ev[:])

    nc.compile()
    _CACHE["nc"] = nc
    return nc


def _prep_core_inputs(hs_b, pos_row_b, pos_col_b, q_w, q_b, k_w, k_b, v_w,
                      rel_table, o_w, g):
    gsl = slice(g * GD, (g + 1) * GD)
    hsT = np.ascontiguousarray(hs_b.T.astype(np.float32))
    wqT = np.ascontiguousarray((q_w[gsl, :] * SCALING).T.astype(np.float32))
    wkT = np.ascontiguousarray(k_w[gsl, :].T.astype(np.float32))
    wvT = np.ascontiguousarray(v_w[gsl, :].T.astype(np.float32))
    woT = np.ascontiguousarray(o_w[:, gsl].T.astype(np.float32))
    pr = np.asarray(pos_row_b).astype(np.int64)
    pc = np.asarray(pos_col_b).astype(np.int64)
    rpt = np.zeros((RP, S), np.float32)
    rpt[pr, np.arange(S)] = 1.0
    rpt[NROW + pc, np.arange(S)] = 1.0
    eye = np.eye(128, dtype=np.float32)
    qb = np.ascontiguousarray(
        (q_b[gsl] * SCALING).astype(np.float32).reshape(MT, 128).T)
    kb = np.ascontiguousarray(k_b[gsl].astype(np.float32).reshape(MT, 128).T)
    t = rel_table[:, g * HPG:(g + 1) * HPG].astype(np.float32)  # [4, 6]
    c0v, c1v = t[0], t[1] - t[0]
    c2v, c3v = t[2] - t[0], t[3] - t[1] - t[2] + t[0]
    c12 = np.zeros((RP, HPG), np.float32)
    c12[0:NROW, :] = c1v[None, :]
    c12[NROW:RP, :] = c2v[None, :]
    c3m = np.broadcast_to(c3v[None, :], (128, HPG)).astype(np.float32).copy()
    c0m = np.broadcast_to(c0v[None, :], (128, HPG)).astype(np.float32).copy()

    return {
        "hsT": hsT, "wqT": wqT, "wkT": wkT, "wvT": wvT, "woT": woT,
        "rpt": rpt.astype(ml_dtypes.bfloat16),
        "eye": eye.astype(ml_dtypes.bfloat16),
        "qb": qb, "kb": kb, "c12": c12, "c3": c3m, "c0": c0m,
    }


def make_in_maps(hidden_states, pos_row, pos_col, q_w, q_b, k_w, k_b, v_w,
                 rel_table, o_w):
    in_maps = []
    for c in range(8):
        b, g = c // HG, c % HG
        in_maps.append(_prep_core_inputs(
            hidden_states[b], pos_row[b], pos_col[b], q_w, q_b, k_w, k_b,
            v_w, rel_table, o_w, g))
    return in_maps


def assemble(results, v_b, o_w, o_b):
    # v_b contributes exactly v_b @ o_w_g.T per group (softmax rows sum to 1)
    bias_row = o_b.copy()
    for g in range(HG):
        gsl = slice(g * GD, (g + 1) * GD)
        bias_row = bias_row + v_b[gsl] @ o_w[:, gsl].T
    out = np.empty((B, S, E), np.float32)
    for b in range(B):
        out[b] = (results[2 * b]["outp"] + results[2 * b + 1]["outp"]
                  + bias_row[None, :])
    return out


def kernel(hidden_states, pos_row, pos_col, q_w, q_b, k_w, k_b, v_w, v_b,
           o_w, o_b, rel_table):
    hidden_states = np.asarray(hidden_states, dtype=np.float32)
    q_w = np.asarray(q_w, dtype=np.float32); q_b = np.asarray(q_b, dtype=np.float32)
    k_w = np.asarray(k_w, dtype=np.float32); k_b = np.asarray(k_b, dtype=np.float32)
    v_w = np.asarray(v_w, dtype=np.float32); v_b = np.asarray(v_b, dtype=np.float32)
    o_w = np.asarray(o_w, dtype=np.float32); o_b = np.asarray(o_b, dtype=np.float32)
    rel_table = np.asarray(rel_table, dtype=np.float32)

    nc = build_nc()
    in_maps = make_in_maps(hidden_states, pos_row, pos_col, q_w, q_b, k_w,
                           k_b, v_w, rel_table, o_w)
    res = run_bass_kernel_spmd(nc, in_maps, core_ids=list(range(8)))
    return assemble(res.results, v_b, o_w, o_b)
